# revision 1
# baseline (speedup 1.0000x reference)
"""DeepISP Trainium2 Bass kernel — 8-core SPMD, H-sharded with halo
redundancy, fold-2 row layout, bf16 matmuls with fp32 PSUM accumulation.

Sharding: core c owns output rows [64c, 64c+64). Local row l <-> global
64c - 12 + l, l in [0, 88). All full-res convs are computed per-core on
the halo-extended slice (no inter-layer communication); the high-level
path runs sharded down to pool2, then one AllGather replicates pool2 so
every core computes the tiny h3/gap/linear tail and the 3x10 color
matrix locally; the per-pixel quadratic Tform is applied to owned rows.

Fold-2 layout with Z channel order: activation buffers are
[128 partitions, 44 super-rows, 514 cols] bf16, super-row s holds image
rows (2s, 2s+1), cols 0/513 zero pads. Partition slots (Z order):
  0:61  rh channels 0..60 of the even row
  61:64 lh channels 61..63 of the even row
  64:67 lh channels 61..63 of the odd row
  67:128 rh channels 0..60 of the odd row
so even-row slots are 0:64 and odd-row slots are 64:128 (contiguous),
relu covers one partition-contiguous op and tanh another.

A 3x3 conv is, per output super-row j and kw in {0,1,2}, two dense
[128x128] matmuls accumulating into one PSUM bank [128, 512]:
  dense rhs = buf[:, j,   kw:kw+512]   (in-sr taps, kh = ip-op+1)
  cross rhs = S[:, j, kw:kw+512] where S[0:64,j] = buf[0:64,j+1]
              (next even row) and S[64:128,j] = buf[64:128,j-1]
              (prev odd row); weights route kh=2 / kh=0 taps.
S is built batch-by-batch from the writeback output (even half on DVE,
odd half on GPSIMD), so the PE runs 6 matmul slots per super-row
instead of 9 (dense+quads) and every slot is a full 128x128.
"""
import os
import sys

for _p in ("/opt/trn_rl_repo", "/root/.axon_site/_ro/trn_rl_repo"):
    if os.path.isdir(_p) and _p not in sys.path:
        sys.path.insert(0, _p)

import numpy as np
import ml_dtypes
from contextlib import ExitStack

import concourse.bass as bass
from concourse import bacc
import concourse.mybir as mybir
import concourse.tile as tile
from concourse.bass_utils import run_bass_kernel_spmd

bf16 = mybir.dt.bfloat16
f32 = mybir.dt.float32
AF = mybir.ActivationFunctionType
ALU = mybir.AluOpType
nbf = ml_dtypes.bfloat16

NCORES = 8
H = W = 512
HALO = 12          # local row 0 = global 64c-12
LR = 88
NSR = 44
SLAB = 514
BATCH = 4

R_I0 = (1, 42)
R_LL = [(1, 41), (2, 41), (2, 40), (3, 40)]
R_HL0 = (3, 39)
R_H1 = (2, 19)
ZT, ZB = 5, 38
OWN0 = 6           # owned output super-rows 6..37
NOWN = 32

FEAT_PERM = list(range(10))

_cached = {}


def _batches(lo, hi, bsz=BATCH):
    out, j = [], lo
    while j <= hi:
        out.append(list(range(j, min(j + bsz, hi + 1))))
        j += bsz
    return out


def _ap(obj, d_part, extra_free, dims):
    """Custom AP anchored at (partition d_part, free elem offset) of an
    AP/tile view. dims = [[step,count],...] in elements (partition dim
    first, step in per-partition-element units for SBUF)."""
    a = obj[:] if hasattr(obj, "tile_context") or not isinstance(obj, bass.AP) else obj
    pstep = a.ap[0][0]
    return bass.AP(a.tensor, a.offset + d_part * pstep + extra_free,
                   [[dims[0][0] * pstep, dims[0][1]]] + list(dims[1:]))


def _flat(tile_obj, p0, pn, sr0, nsr):
    """Flat 2D AP [pn parts, nsr*SLAB] over a [128, NSR, SLAB] tile --
    keeps DVE copies in the fast (2D dense) path."""
    a = tile_obj[:]
    return bass.AP(a.tensor, a.offset + p0 * a.ap[0][0] + sr0 * SLAB,
                   [[a.ap[0][0], pn], [1, nsr * SLAB]])


# ---------------------------------------------------------------------------
# host-side weight packing
# ---------------------------------------------------------------------------

def _zmap(p):
    """Z slot -> (row parity, channel).
    even block [0:64):  rh 0..31 at 0:32, lh 61..63 at 32:35, rh 32..60
    at 35:64; odd block [64:128): lh 61..63 at 64:67, rh 0..60 at 67:128.
    lh slots sit at 32/64-aligned starts so the tanh PSUM reads are legal
    exact segments."""
    if p < 32:
        return 0, p
    if p < 35:
        return 0, 61 + (p - 32)
    if p < 64:
        return 0, p - 3
    if p < 67:
        return 1, 61 + (p - 64)
    return 1, p - 67


def _smap(p):
    return p // 64, p % 64


def _pack_block(wfull, in_map, out_map, kind):
    """[128, 3*128] dense or cross weight block. wfull [oc, ic, kh, kw]."""
    out = np.zeros((128, 3 * 128), np.float32)
    for kw in range(3):
        blk = np.zeros((128, 128), np.float32)
        for p in range(128):
            ip, ic = in_map(p)
            ipe = ip if kind == "dense" else (2 if ip == 0 else -1)
            for o in range(128):
                op, oc = out_map(o)
                kh = ipe - op + 1
                if 0 <= kh <= 2:
                    blk[p, o] = wfull[oc, ic, kh, kw]
        out[:, kw * 128:(kw + 1) * 128] = blk
    return out


def _pack_h1(w):
    """h1 stride-2 dense+B+C packing (std layout), as in baseline."""
    dense = np.zeros((128, 3 * 128), np.float32)
    for kw in range(3):
        blk = np.zeros((128, 128), np.float32)
        for a in range(2):
            for b in range(2):
                kh = a - 2 * b + 1
                if 0 <= kh <= 2:
                    blk[a * 64:(a + 1) * 64, b * 64:(b + 1) * 64] = \
                        w[:, :, kh, kw].T
        dense[:, kw * 128:(kw + 1) * 128] = blk
    B = np.zeros((128, 3 * 64), np.float32)
    C = np.zeros((128, 3 * 64), np.float32)
    for kw in range(3):
        B[64:128, kw * 64:(kw + 1) * 64] = w[:, :, 0, kw].T
        C[0:64, kw * 64:(kw + 1) * 64] = w[:, :, 1, kw].T
        C[64:128, kw * 64:(kw + 1) * 64] = w[:, :, 2, kw].T
    return np.concatenate([dense, B, C], 1)      # [128, 768]


def _pack_im2col_w(w):
    out = np.zeros((64, 9 * 64), np.float32)
    for t in range(9):
        kh, kw = divmod(t, 3)
        out[:, t * 64:(t + 1) * 64] = w[:, :, kh, kw].T
    return out


def _pack_weights(inp):
    pk = {}
    # low0: [54, 128], rows 27q+3t+ch (q = out-row parity), cols Z slots
    w54 = np.zeros((54, 128), np.float32)
    for o in range(128):
        op, oc = _zmap(o)
        for t in range(9):
            kh, kw = divmod(t, 3)
            for ch in range(3):
                w54[27 * op + 3 * t + ch, o] = inp["low0_w"][oc, ch, kh, kw]
    pk["w_low0"] = w54

    main = []
    for i in range(4):
        wf = np.zeros((64, 64, 3, 3), np.float32)
        wf[:61, :61] = inp["ll_rh_w"][i]
        wf[61:, 61:] = inp["ll_lh_w"][i]
        main.append(np.concatenate(
            [_pack_block(wf, _zmap, _zmap, "dense"),
             _pack_block(wf, _zmap, _zmap, "cross")], 1))
    whl0 = np.zeros((64, 64, 3, 3), np.float32)
    whl0[:, :61] = inp["hl0_w"]
    main.append(np.concatenate(
        [_pack_block(whl0, _zmap, _smap, "dense"),
         _pack_block(whl0, _zmap, _smap, "cross")], 1))
    main.append(_pack_h1(inp["hl_w"][0]))
    pk["w_main"] = np.concatenate(main, 1)       # [128, 6*768]

    pk["w_h2h3"] = np.concatenate(
        [_pack_im2col_w(inp["hl_w"][1]), _pack_im2col_w(inp["hl_w"][2])], 1)
    pk["w_lin"] = (inp["lin_w"].T / 64.0).astype(np.float32)
    selL = np.zeros((30, 20), np.float32)
    for q in range(30):
        for p in range(20):
            if q % 10 == FEAT_PERM[p % 10]:
                selL[q, p] = 1.0
    pk["w_sel"] = selL
    cmask = np.zeros((30, 6), np.float32)
    for q in range(30):
        for n in range(6):
            if q // 10 == n % 3:
                cmask[q, n] = 1.0
    pk["cmask"] = cmask
    pmask = np.zeros((20, 6), np.float32)
    for p in range(20):
        for n in range(6):
            if p // 10 == n // 3:
                pmask[p, n] = 1.0
    pk["pmask"] = pmask

    bias = np.zeros((128, 9), np.float32)
    llb = [np.concatenate([inp["ll_rh_b"][i], inp["ll_lh_b"][i]])
           for i in range(4)]
    for p in range(128):
        _, c = _zmap(p)
        bias[p, 0] = inp["low0_b"][c]
        for i in range(4):
            bias[p, 1 + i] = llb[i][c]
    bias[0:64, 5] = bias[64:128, 5] = inp["hl0_b"]
    bias[0:64, 6] = bias[64:128, 6] = inp["hl_b"][0]
    bias[0:64, 7] = inp["hl_b"][1]
    bias[0:64, 8] = inp["hl_b"][2]
    pk["bias"] = bias
    pk["lin_b"] = inp["lin_b"].reshape(30, 1).astype(np.float32)
    return pk


# ---------------------------------------------------------------------------
# device program
# ---------------------------------------------------------------------------

def _emit_fold_layer(nc, psum, src, S, dst, wts, bcol, bias_sb, mask_sb,
                     rng, kind, build_S):
    wd = [wts[:, k * 128:(k + 1) * 128] for k in range(3)]
    wx = [wts[:, 384 + k * 128:384 + (k + 1) * 128] for k in range(3)]
    odd_pending = None
    for batch in _batches(*rng):
        accs = [psum.tile([128, 512], f32, name=f"acc{i}", tag=f"b{i}")
                for i in range(len(batch))]
        for kw in range(3):
            for i, j in enumerate(batch):
                nc.tensor.matmul(accs[i][:], wd[kw], src[:, j, kw:kw + 512],
                                 start=(kw == 0), stop=False)
        for kw in range(3):
            for i, j in enumerate(batch):
                nc.tensor.matmul(accs[i][:], wx[kw], S[:, j, kw:kw + 512],
                                 start=False, stop=(kw == 2))
        for i, j in enumerate(batch):
            acc, be = accs[i], bias_sb[:, bcol:bcol + 1]
            if kind == "ll":
                # relu+bias on all 128 slots; the lh slots (32:35, 64:67,
                # both at legal aligned PSUM starts) are then overwritten
                # with exact tanh segments.
                nc.vector.tensor_scalar(dst[:, j, 1:513], acc[:],
                                        be, 0.0, ALU.add, ALU.max)
                nc.scalar.activation(dst[32:35, j, 1:513], acc[32:35, :],
                                     AF.Tanh, bias=be[32:35])
                nc.scalar.activation(dst[64:67, j, 1:513], acc[64:67, :],
                                     AF.Tanh, bias=be[64:67])
            else:
                nc.vector.tensor_scalar(dst[0:64, j, 1:513], acc[0:64, :],
                                        be[0:64], None, ALU.add)
                nc.scalar.activation(dst[64:128, j, 1:513], acc[64:128, :],
                                     AF.Identity, bias=be[64:128])
            if j <= ZT or j >= ZB:
                nc.vector.tensor_scalar_mul(dst[:, j, 1:513], dst[:, j, 1:513],
                                            mask_sb[:, j:j + 1])
        if build_S:
            # Even half writes backward (b0-1..): never clobbers an S entry
            # a later cross matmul of THIS layer still needs. The odd half
            # writes forward (b0+1..b0+n), overlapping the next batch's
            # cross reads -- so it is emitted one batch late, after those
            # reads are in program order.
            b0, n = batch[0], len(batch)
            nc.vector.tensor_copy(_flat(S, 0, 64, b0 - 1, n),
                                  _flat(dst, 0, 64, b0, n))
            if odd_pending is not None:
                p0, pn = odd_pending
                nc.vector.tensor_copy(_flat(S, 64, 64, p0 + 1, pn),
                                      _flat(dst, 64, 64, p0, pn))
            odd_pending = (b0, n)
    if build_S and odd_pending is not None:
        p0, pn = odd_pending
        nc.vector.tensor_copy(_flat(S, 64, 64, p0 + 1, pn),
                              _flat(dst, 64, 64, p0, pn))


def _emit_feats(nc, bufA, xT, yT, featsT, half):
    """Build feats for one 16-sr half (half in {0,1}) of the owned region,
    in two 8-sr sub-chunks, into featsT [20, 16*SLAB]. xT/yT are shared
    [20, 8*SLAB] scratch; constant rows are pre-set to 1.0 on half 0."""
    if half == 0:
        nc.vector.memset(xT[:], 1.0)
        nc.vector.memset(yT[:], 1.0)
    for sub in range(2):
        s0 = OWN0 + half * 16 + sub * 8
        XRUNS = [(0, 4), (4, 3), (7, 2)]          # r,g,b -> X row runs
        YRUNS = [[0], [1, 4], [2, 5, 7]]          # r,g,b -> Y rows
        for par in range(2):
            ba = bufA[:]
            for ci in range(3):
                sp = (32 if par == 0 else 64) + ci  # Z lh slots
                soff = ba.offset + sp * ba.ap[0][0] + s0 * SLAB
                p0, n = XRUNS[ci]
                srcap = bass.AP(ba.tensor, soff,
                                [[ba.ap[0][0], 1], [0, n], [1, 8 * SLAB]])
                dst = _ap(xT, 10 * par + p0, 0, [[1, n], [1, 8 * SLAB]])
                nc.sync.dma_start(dst, srcap)
                for yp in YRUNS[ci]:
                    srcy = bass.AP(ba.tensor, soff,
                                   [[ba.ap[0][0], 1], [1, 8 * SLAB]])
                    dsty = _ap(yT, 10 * par + yp, 0, [[1, 1], [1, 8 * SLAB]])
                    nc.sync.dma_start(dsty, srcy)
        nc.vector.tensor_mul(featsT[:, sub * 8 * SLAB:(sub + 1) * 8 * SLAB],
                             xT[:], yT[:])


def _emit_tform_half(nc, psum, wm20, featsT, outstage, out_d, half):
    for sub in range(2):
        srs = [OWN0 + half * 16 + sub * 8 + i for i in range(8)]
        ost = outstage
        for bi, bt in enumerate([srs[0:4], srs[4:8]]):
            accs = [psum.tile([6, 512], f32, name=f"acc{i}", tag=f"b{i}")
                    for i in range(len(bt))]
            for i, j in enumerate(bt):
                fo = (j - OWN0 - half * 16) * SLAB + 1
                nc.tensor.matmul(accs[i][:], wm20[:], featsT[:, fo:fo + 512],
                                 start=True, stop=True)
            for i, j in enumerate(bt):
                so = (j - srs[0]) * 512
                nc.scalar.activation(ost[:, so:so + 512], accs[i][:], AF.Copy)
        r0 = 2 * (srs[0] - OWN0)
        for par in range(2):
            sap = _ap(ost, par * 3, 0, [[1, 3], [512, 8], [1, 512]])
            dap = bass.AP(out_d[:].tensor, (r0 + par) * 512,
                          [[64 * 512, 3], [2 * 512, 8], [1, 512]])
            nc.sync.dma_start(dap, sap)


def _build_program(debug=False):
    nc = bacc.Bacc("TRN2", target_bir_lowering=False, debug=False,
                   num_devices=NCORES)

    x_in = nc.dram_tensor("x", [3, LR, SLAB], bf16, kind="ExternalInput")
    wmain_in = nc.dram_tensor("w_main", [128, 6 * 768], bf16,
                              kind="ExternalInput")
    wlow0_in = nc.dram_tensor("w_low0", [54, 128], bf16, kind="ExternalInput")
    wh23_in = nc.dram_tensor("w_h2h3", [64, 1152], bf16, kind="ExternalInput")
    wsel_in = nc.dram_tensor("w_sel", [30, 20], bf16, kind="ExternalInput")
    cmask_in = nc.dram_tensor("cmask", [30, 6], f32, kind="ExternalInput")
    pmask_in = nc.dram_tensor("pmask", [20, 6], f32, kind="ExternalInput")
    wlin_in = nc.dram_tensor("w_lin", [64, 30], f32, kind="ExternalInput")
    bias_in = nc.dram_tensor("bias", [128, 9], f32, kind="ExternalInput")
    linb_in = nc.dram_tensor("lin_b", [30, 1], f32, kind="ExternalInput")
    maskI_in = nc.dram_tensor("mask_i", [128, NSR], f32, kind="ExternalInput")
    maskh1_in = nc.dram_tensor("mask_h1", [128, 20], f32, kind="ExternalInput")

    out_d = nc.dram_tensor("out", [3, 64, W], f32, kind="ExternalOutput")

    wp_dram = nc.dram_tensor("wp_dram", [30], f32)  # noqa: F841
    cc_in = nc.dram_tensor("cc_in", [64 * 4 * 32], bf16)
    cc_gath = nc.dram_tensor("cc_gath", [NCORES * 64 * 4 * 32], bf16,
                             addr_space="Shared")
    dbg = {}
    if debug:
        for nm, shp, dt in [("i0", [128, NSR * SLAB], bf16),
                            ("i1", [128, NSR * SLAB], bf16),
                            ("i2", [128, NSR * SLAB], bf16),
                            ("i3", [128, NSR * SLAB], bf16),
                            ("i4", [128, NSR * SLAB], bf16),
                            ("hl0", [128, NSR * SLAB], bf16),
                            ("h1", [128, 18 * 256], bf16),
                            ("pool1", [64, 20 * 130], bf16),
                            ("pool2", [64, 4 * 32], bf16),
                            ("wp", [30, 1], f32)]:
            dbg[nm] = nc.dram_tensor("dbg_" + nm, shp, dt, kind="ExternalOutput")

    with tile.TileContext(nc) as tc, ExitStack() as ctx:
        pers = ctx.enter_context(tc.tile_pool(name="pers", bufs=1))
        psum = ctx.enter_context(tc.tile_pool(name="psum", bufs=2, space="PSUM"))

        # persistent tiles
        w_low0 = pers.tile([54, 128], bf16)
        bias_sb = pers.tile([128, 9], f32)
        mask_sb = pers.tile([128, NSR], f32)
        w_main = pers.tile([128, 6 * 768], bf16)
        bufA = pers.tile([128, NSR, SLAB], bf16)
        bufB = pers.tile([128, NSR, SLAB], bf16)
        w_h2h3 = pers.tile([64, 1152], bf16)
        w_sel = pers.tile([30, 20], bf16)
        cmask_sb = pers.tile([30, 6], f32)
        pmask_sb = pers.tile([20, 6], f32)
        w_lin = pers.tile([64, 30], f32)
        linb_sb = pers.tile([30, 1], f32)
        maskh1_sb = pers.tile([128, 20], f32)

        sp_cm = tc.tile_pool(name="sp", bufs=1)
        sp = sp_cm.__enter__()
        Sbuf = sp.tile([128, NSR, SLAB], bf16)

        # ---- low0 via fold-K im2col: one [54->128] matmul per super-row.
        # Emit chunk-0 input DMAs first so the PE can start ASAP.
        with tc.tile_pool(name="imcp", bufs=2) as imcp:
            first = True
            for j0 in range(R_I0[0], R_I0[1] + 1, 11):
                ns = min(11, R_I0[1] + 1 - j0)
                imc = imcp.tile([54, 11 * 512], bf16, name="imc", tag="imc")
                for q in range(2):
                    for t in range(9):
                        kh, kw = divmod(t, 3)
                        src = bass.AP(x_in[:].tensor,
                                      (2 * j0 + q - 1 + kh) * SLAB + kw,
                                      [[LR * SLAB, 3], [2 * SLAB, ns], [1, 512]])
                        nc.sync.dma_start(
                            imc[27 * q + 3 * t:27 * q + 3 * t + 3, 0:ns * 512],
                            src)
                if first:
                    # weight/bias/mask loads after the first imc chunk
                    nc.sync.dma_start(w_low0[:], wlow0_in[:])
                    nc.sync.dma_start(bias_sb[:], bias_in[:])
                    nc.sync.dma_start(mask_sb[:], maskI_in[:])
                    nc.sync.dma_start(w_main[:], wmain_in[:])
                    nc.sync.dma_start(w_h2h3[:], wh23_in[:])
                    nc.sync.dma_start(w_sel[:], wsel_in[:])
                    nc.sync.dma_start(cmask_sb[:], cmask_in[:])
                    nc.sync.dma_start(pmask_sb[:], pmask_in[:])
                    nc.sync.dma_start(w_lin[:], wlin_in[:])
                    nc.sync.dma_start(linb_sb[:], linb_in[:])
                    nc.sync.dma_start(maskh1_sb[:], maskh1_in[:])
                    # zero pads (cols 0/513) and S edge super-rows
                    nc.gpsimd.memset(bufA[:, :, 0:1], 0.0)
                    nc.gpsimd.memset(bufA[:, :, 513:514], 0.0)
                    nc.gpsimd.memset(bufB[:, :, 0:1], 0.0)
                    nc.gpsimd.memset(bufB[:, :, 513:514], 0.0)
                    nc.gpsimd.memset(Sbuf[64:128, 0:2, :], 0.0)
                    first = False
                for bt in _batches(j0, j0 + ns - 1):
                    accs = [psum.tile([128, 512], f32, name=f"acc{i}",
                                      tag=f"b{i}") for i in range(len(bt))]
                    for i, j in enumerate(bt):
                        si = j - j0
                        nc.tensor.matmul(accs[i][:], w_low0[:],
                                         imc[:, si * 512:(si + 1) * 512],
                                         start=True, stop=True)
                    for i, j in enumerate(bt):
                        be = bias_sb[:, 0:1]
                        if j % 2 == 0:
                            nc.vector.tensor_scalar(bufA[:, j, 1:513],
                                                    accs[i][:], be,
                                                    None, ALU.add)
                        else:
                            nc.scalar.activation(bufA[:, j, 1:513],
                                                 accs[i][:], AF.Identity,
                                                 bias=be)
                        if j <= ZT or j >= ZB:
                            nc.vector.tensor_scalar_mul(
                                bufA[:, j, 1:513], bufA[:, j, 1:513],
                                mask_sb[:, j:j + 1])
                    b0, n = bt[0], len(bt)
                    nc.vector.tensor_copy(_flat(Sbuf, 0, 64, b0 - 1, n),
                                          _flat(bufA, 0, 64, b0, n))
                    nc.vector.tensor_copy(_flat(Sbuf, 64, 64, b0 + 1, n),
                                          _flat(bufA, 64, 64, b0, n))

        if debug:
            nc.sync.dma_start(dbg["i0"][:],
                              bufA[:].rearrange("p a b -> p (a b)"))

        # ---- ll layers + hl0 (S-trick fold layers) ----
        bufs = [bufA, bufB]
        for i in range(4):
            if i == 1:
                # ll1 reads S[0:64, 41] which must be zero (bufB sr 42
                # was never written); low0's build left stale data there.
                nc.gpsimd.memset(Sbuf[0:64, 41:42, :], 0.0)
            _emit_fold_layer(nc, psum, bufs[i % 2], Sbuf, bufs[(i + 1) % 2],
                             w_main[:, i * 768:(i + 1) * 768], 1 + i,
                             bias_sb, mask_sb, R_LL[i], "ll", True)
            if debug and i < 3:
                nc.sync.dma_start(
                    dbg[f"i{i + 1}"][:],
                    bufs[(i + 1) % 2][:].rearrange("p a b -> p (a b)"))
        _emit_fold_layer(nc, psum, bufA, Sbuf, bufB,
                         w_main[:, 4 * 768:5 * 768], 5,
                         bias_sb, mask_sb, R_HL0, "copy", False)
        sp_cm.__exit__(None, None, None)
        if debug:
            nc.sync.dma_start(dbg["i4"][:],
                              bufA[:].rearrange("p a b -> p (a b)"))
            nc.sync.dma_start(dbg["hl0"][:],
                              bufB[:].rearrange("p a b -> p (a b)"))

        with tc.tile_pool(name="hlp", bufs=1) as hlp:
            # ---- h1 (stride-2 fold conv from bufB, std layout) ----
            wh1 = w_main[:, 5 * 768:]
            wA = [wh1[:, k * 128:(k + 1) * 128] for k in range(3)]
            wB = [wh1[:, 384 + k * 64:384 + (k + 1) * 64] for k in range(3)]
            wC = [wh1[:, 576 + k * 64:576 + (k + 1) * 64] for k in range(3)]
            h1fold = hlp.tile([128, 18, 256], bf16)
            for batch in _batches(*R_H1):
                accs = [psum.tile([128, 256], f32, name=f"acc{i}",
                                  tag=f"b{i}") for i in range(len(batch))]
                for kw in range(3):
                    for i, m in enumerate(batch):
                        nc.tensor.matmul(accs[i][:], wA[kw],
                                         bufB[:, 2 * m, kw:kw + 512:2],
                                         start=(kw == 0), stop=False)
                for kw in range(3):
                    for i, m in enumerate(batch):
                        nc.tensor.matmul(accs[i][0:64, :], wB[kw][64:128, :],
                                         bufB[64:128, 2 * m - 1, kw:kw + 512:2],
                                         start=False, stop=False,
                                         tile_position=(64, 0))
                    for i, m in enumerate(batch):
                        nc.tensor.matmul(accs[i][64:128, :], wC[kw][:],
                                         bufB[:, 2 * m + 1, kw:kw + 512:2],
                                         start=False, stop=(kw == 2),
                                         tile_position=(0, 64))
                for i, m in enumerate(batch):
                    be = bias_sb[:, 6:7]
                    sl = h1fold[:, m - 2, :]
                    nc.vector.tensor_scalar(sl[0:64, :], accs[i][0:64, :],
                                            be[0:64], 0.0, ALU.add, ALU.max)
                    nc.scalar.activation(sl[64:128, :], accs[i][64:128, :],
                                         AF.Relu, bias=be[64:128])
                    if m in (2, 3, 18, 19):
                        nc.vector.tensor_scalar_mul(sl[:], sl[:],
                                                    maskh1_sb[:, m:m + 1])
            if debug:
                nc.sync.dma_start(dbg["h1"][:],
                                  h1fold[:].rearrange("p a b -> p (a b)"))

            # pool1 via in-SBUF 2x2 max: horizontal pairs then the two
            # parity halves (partition-shifted operands).
            h1h = hlp.tile([128, 18, 128], bf16)
            nc.vector.tensor_max(h1h[:], h1fold[:, :, 0:256:2],
                                 h1fold[:, :, 1:256:2])
            h1v = hlp.tile([64, 18, 128], bf16)
            _h1h = h1h[:]
            nc.vector.tensor_copy(
                bass.AP(h1v[:].tensor, h1v[:].offset,
                        [[h1v[:].ap[0][0], 64], [1, 18 * 128]]),
                bass.AP(_h1h.tensor, _h1h.offset + 64 * _h1h.ap[0][0],
                        [[_h1h.ap[0][0], 64], [1, 18 * 128]]))
            pool1 = hlp.tile([64, 20, 130], bf16)
            nc.gpsimd.memset(pool1[:], 0.0)
            nc.vector.tensor_max(pool1[:, 2:20, 1:129],
                                 h1h[0:64, :, :], h1v[:])
            if debug:
                nc.sync.dma_start(dbg["pool1"][:],
                                  pool1[:].rearrange("p a b -> p (a b)"))

            # ---- h2 via im2col (9 taps, K=64) ----
            imc2 = hlp.tile([64, 9 * 512], bf16)
            for t in range(9):
                kh, kw = divmod(t, 3)
                src = _ap(pool1, 0, (2 + kh) * 130 + kw,
                          [[1, 64], [2 * 130, 8], [2, 64]])
                if t % 2 == 0:
                    nc.vector.tensor_copy(imc2[:, t * 512:(t + 1) * 512], src)
                else:
                    nc.scalar.activation(imc2[:, t * 512:(t + 1) * 512], src,
                                         AF.Copy)
            acc2 = psum.tile([64, 512], f32, name="acc0", tag="b0")
            for t in range(9):
                nc.tensor.matmul(acc2[:], w_h2h3[:, t * 64:(t + 1) * 64],
                                 imc2[:, t * 512:(t + 1) * 512],
                                 start=(t == 0), stop=(t == 8))
            h2sb = hlp.tile([64, 8, 64], bf16)
            nc.scalar.activation(h2sb[:].rearrange("p a b -> p (a b)"),
                                 acc2[:], AF.Relu, bias=bias_sb[0:64, 7:8])

            # pool2 -> cc_in
            tmp2 = hlp.tile([64, 8, 32], bf16)
            nc.vector.tensor_max(tmp2[:], h2sb[:, :, 0:64:2],
                                 h2sb[:, :, 1:64:2])
            pool2 = hlp.tile([64, 4, 32], bf16)
            nc.vector.tensor_max(pool2[:], tmp2[:, 0:8:2, :],
                                 tmp2[:, 1:8:2, :])
            nc.sync.dma_start(cc_in[:],
                              pool2[:].rearrange("p a b -> p (a b)"))
            if debug:
                nc.sync.dma_start(dbg["pool2"][:],
                                  pool2[:].rearrange("p a b -> p (a b)"))

        with tc.tile_pool(name="tfp", bufs=1) as tfp:
            featsT = [tfp.tile([20, 16 * SLAB], bf16, name=f"featsT{h}")
                      for h in range(2)]
            xT = tfp.tile([20, 8 * SLAB], bf16)
            yT = tfp.tile([20, 8 * SLAB], bf16)
            outstage = tfp.tile([6, 8 * 512], f32)
            wm20 = tfp.tile([20, 6], bf16)

            # p2f cleared ahead of time; feats DMAs/muls run during the
            # collective (they only touch bufA / DVE / sync queues).
            p2f = tfp.tile([64, 34, 34], bf16)
            nc.gpsimd.memset(p2f[:], 0.0)
            _emit_feats(nc, bufA, xT, yT, featsT[0], 0)
            _emit_feats(nc, bufA, xT, yT, featsT[1], 1)

            # ---- AllGather pool2 ----
            with tc.tile_critical():
                cc_sem = nc.alloc_semaphore("cc_sem")
                nc.gpsimd.collective_compute(
                    "AllGather", ALU.bypass,
                    replica_groups=[list(range(NCORES))],
                    ins=[cc_in[:]], outs=[cc_gath[:]],
                ).then_inc(cc_sem)
                nc.gpsimd.wait_ge(cc_sem, 1)

            # ---- h3 tail (replicated) ----
            for q in range(NCORES):
                src = bass.AP(cc_gath[:].tensor, q * 64 * 4 * 32,
                              [[4 * 32, 64], [32, 4], [1, 32]])
                nc.sync.dma_start(p2f[:, 1 + 4 * q:5 + 4 * q, 1:33], src)
            imc3 = tfp.tile([64, 9 * 256], bf16)
            for t in range(9):
                kh, kw = divmod(t, 3)
                src = _ap(p2f, 0, kh * 34 + kw,
                          [[1, 64], [2 * 34, 16], [2, 16]])
                if t % 2 == 0:
                    nc.vector.tensor_copy(imc3[:, t * 256:(t + 1) * 256], src)
                else:
                    nc.scalar.activation(imc3[:, t * 256:(t + 1) * 256], src,
                                         AF.Copy)
            acc3 = psum.tile([64, 256], f32, name="acc1", tag="b1")
            for t in range(9):
                nc.tensor.matmul(acc3[:],
                                 w_h2h3[:, 576 + t * 64:576 + (t + 1) * 64],
                                 imc3[:, t * 256:(t + 1) * 256],
                                 start=(t == 0), stop=(t == 8))
            h3sb = tfp.tile([64, 16, 16], bf16)
            nc.scalar.activation(h3sb[:].rearrange("p a b -> p (a b)"),
                                 acc3[:], AF.Relu, bias=bias_sb[0:64, 8:9])
            tmp3 = tfp.tile([64, 16, 8], bf16)
            nc.vector.tensor_max(tmp3[:], h3sb[:, :, 0:16:2],
                                 h3sb[:, :, 1:16:2])
            h3p = tfp.tile([64, 8, 8], f32)
            nc.vector.tensor_max(h3p[:], tmp3[:, 0:16:2, :],
                                 tmp3[:, 1:16:2, :])
            gsum = tfp.tile([64, 1], f32)
            nc.vector.reduce_sum(gsum[:],
                                 h3p[:].rearrange("p a b -> p (a b)"),
                                 axis=mybir.AxisListType.X)
            accW = psum.tile([30, 1], f32, name="acc2", tag="b2")
            nc.tensor.matmul(accW[:], w_lin[:], gsum[:],
                             start=True, stop=True)
            wp_sb = tfp.tile([30, 1], f32)
            nc.scalar.activation(wp_sb[:], accW[:], AF.Identity,
                                 bias=linb_sb[:])
            if debug:
                nc.sync.dma_start(dbg["wp"][:], wp_sb[:])
            wpR = tfp.tile([30, 6], bf16)
            nc.vector.tensor_scalar_mul(wpR[:], cmask_sb[:], wp_sb[:])
            accM = psum.tile([20, 6], f32, name="acc3", tag="b3")
            nc.tensor.matmul(accM[:], w_sel[:], wpR[:],
                             start=True, stop=True)
            nc.vector.tensor_tensor(wm20[:], accM[:], pmask_sb[:],
                                    ALU.mult)

            _emit_tform_half(nc, psum, wm20, featsT[0], outstage, out_d, 0)
            _emit_tform_half(nc, psum, wm20, featsT[1], outstage, out_d, 1)

    nc.compile()
    return nc


# ---------------------------------------------------------------------------
# host entry
# ---------------------------------------------------------------------------

def kernel(**inputs):
    inp = {k: np.asarray(v) for k, v in inputs.items()}
    debug = bool(_cached.get("debug", False))
    key = ("nc", debug)
    if key not in _cached:
        _cached[key] = _build_program(debug=debug)
    nc = _cached[key]

    pk = _pack_weights(inp)
    x = np.asarray(inp["x"], np.float32)[0]

    shared = {
        "w_main": pk["w_main"].astype(nbf),
        "w_low0": pk["w_low0"].astype(nbf),
        "w_h2h3": pk["w_h2h3"].astype(nbf),
        "w_sel": pk["w_sel"].astype(nbf),
        "cmask": pk["cmask"],
        "pmask": pk["pmask"],
        "w_lin": pk["w_lin"],
        "bias": pk["bias"],
        "lin_b": pk["lin_b"],
    }
    in_maps = []
    par_col = (np.arange(128) // 64)[:, None]
    for c in range(NCORES):
        g0 = 64 * c - HALO
        xs = np.zeros((3, LR, SLAB), np.float32)
        lo, hi = max(0, -g0), min(LR, H - g0)
        xs[:, lo:hi, 1:513] = x[:, g0 + lo:g0 + hi, :]
        gI = g0 + 2 * np.arange(NSR)[None, :] + par_col
        maskI = ((gI >= 0) & (gI < H)).astype(np.float32)
        gh = 32 * c - 6 + 2 * np.arange(20)[None, :] + par_col
        maskh1 = ((gh >= 0) & (gh < 256)).astype(np.float32)
        im = dict(shared)
        im["x"] = xs.astype(nbf)
        im["mask_i"] = maskI
        im["mask_h1"] = maskh1
        in_maps.append(im)

    res = run_bass_kernel_spmd(nc, in_maps, list(range(NCORES)))
    _cached["last_results"] = res
    out = np.concatenate([res.results[c]["out"] for c in range(NCORES)], axis=1)
    return out[None].astype(np.float32)



# revision 19
# speedup vs baseline: 1.1485x; 1.1485x over previous
"""DeepISP Trainium2 Bass kernel — 8-core SPMD, H-sharded with halo
redundancy, fold-2 row layout, bf16 matmuls with fp32 PSUM accumulation.

Sharding: core c owns output rows [64c, 64c+64). Local row l <-> global
64c - 12 + l, l in [0, 88). All full-res convs are computed per-core on
the halo-extended slice (no inter-layer communication); the high-level
path runs sharded down to pool2, then one AllGather replicates pool2 so
every core computes the tiny h3/gap/linear tail and the 3x10 color
matrix locally; the per-pixel quadratic Tform is applied to owned rows.

Fold-2 layout with Z channel order: activation buffers are
[128 partitions, 44 super-rows, 514 cols] bf16, super-row s holds image
rows (2s, 2s+1), cols 0/513 zero pads. Partition slots (Z order):
  0:61  rh channels 0..60 of the even row
  61:64 lh channels 61..63 of the even row
  64:67 lh channels 61..63 of the odd row
  67:128 rh channels 0..60 of the odd row
so even-row slots are 0:64 and odd-row slots are 64:128 (contiguous),
relu covers one partition-contiguous op and tanh another.

A 3x3 conv is, per output super-row j and kw in {0,1,2}, two dense
[128x128] matmuls accumulating into one PSUM bank [128, 512]:
  dense rhs = buf[:, j,   kw:kw+512]   (in-sr taps, kh = ip-op+1)
  cross rhs = S[:, j, kw:kw+512] where S[0:64,j] = buf[0:64,j+1]
              (next even row) and S[64:128,j] = buf[64:128,j-1]
              (prev odd row); weights route kh=2 / kh=0 taps.

v2 scheduling: per batch of 4 super-rows, the 4 accumulators are the 4
banks of ONE [128, 4, 512] PSUM tile so the relu+bias writeback is a
single DVE op and the two tanh overwrites are single scalar ops over
the whole batch (flat [*, 2048] PSUM reads). The per-pixel feature
tensors are gathered and multiplied right after ll3, overlapping the
hl0/h1/h2 compute, so only h3 + the 3x10 Tform remain after the
AllGather. DMA issue is spread across the sync/scalar HWDGE queues and
the gpsimd SWDGE queue (weight loads) to keep any one queue off the
critical path.
"""
import os
import sys

for _p in ("/opt/trn_rl_repo", "/root/.axon_site/_ro/trn_rl_repo"):
    if os.path.isdir(_p) and _p not in sys.path:
        sys.path.insert(0, _p)

import numpy as np
import ml_dtypes
from contextlib import ExitStack

import concourse.bass as bass
from concourse import bacc
import concourse.mybir as mybir
import concourse.tile as tile
from concourse.bass_utils import run_bass_kernel_spmd

bf16 = mybir.dt.bfloat16
f32 = mybir.dt.float32
AF = mybir.ActivationFunctionType
ALU = mybir.AluOpType
nbf = ml_dtypes.bfloat16

NCORES = 8
H = W = 512
HALO = 12          # local row 0 = global 64c-12
LR = 88
NSR = 44
SLAB = 514
BATCH = 4

R_I0 = (1, 42)
R_LL = [(1, 41), (2, 41), (2, 40), (3, 40)]
R_HL0 = (3, 39)
R_H1 = (2, 19)
ZT, ZB = 5, 38
OWN0 = 6           # owned output super-rows 6..37
NOWN = 32

FEAT_PERM = list(range(10))

_cached = {}


def _batches(lo, hi, bsz=BATCH):
    out, j = [], lo
    while j <= hi:
        out.append(list(range(j, min(j + bsz, hi + 1))))
        j += bsz
    return out


def _ap(obj, d_part, extra_free, dims):
    """Custom AP anchored at (partition d_part, free elem offset) of an
    AP/tile view. dims = [[step,count],...] in elements (partition dim
    first, step in per-partition-element units for SBUF)."""
    a = obj[:] if hasattr(obj, "tile_context") or not isinstance(obj, bass.AP) else obj
    pstep = a.ap[0][0]
    return bass.AP(a.tensor, a.offset + d_part * pstep + extra_free,
                   [[dims[0][0] * pstep, dims[0][1]]] + list(dims[1:]))


def _flat(tile_obj, p0, pn, sr0, nsr):
    """Flat 2D AP [pn parts, nsr*SLAB] over a [128, NSR, SLAB] tile --
    keeps DVE copies in the fast (2D dense) path."""
    a = tile_obj[:]
    return bass.AP(a.tensor, a.offset + p0 * a.ap[0][0] + sr0 * SLAB,
                   [[a.ap[0][0], pn], [1, nsr * SLAB]])


# ---------------------------------------------------------------------------
# host-side weight packing
# ---------------------------------------------------------------------------

def _zmap(p):
    """Z slot -> (row parity, channel).
    even block [0:64):  rh 0..31 at 0:32, lh 61..63 at 32:35, rh 32..60
    at 35:64; odd block [64:128): lh 61..63 at 64:67, rh 0..60 at 67:128.
    lh slots sit at 32/64-aligned starts so the tanh PSUM reads are legal
    exact segments."""
    if p < 32:
        return 0, p
    if p < 35:
        return 0, 61 + (p - 32)
    if p < 64:
        return 0, p - 3
    if p < 67:
        return 1, 61 + (p - 64)
    return 1, p - 67


def _smap(p):
    return p // 64, p % 64


def _pack_block(wfull, in_map, out_map, kind):
    """[128, 3*128] dense or cross weight block. wfull [oc, ic, kh, kw]."""
    out = np.zeros((128, 3 * 128), np.float32)
    for kw in range(3):
        blk = np.zeros((128, 128), np.float32)
        for p in range(128):
            ip, ic = in_map(p)
            ipe = ip if kind == "dense" else (2 if ip == 0 else -1)
            for o in range(128):
                op, oc = out_map(o)
                kh = ipe - op + 1
                if 0 <= kh <= 2:
                    blk[p, o] = wfull[oc, ic, kh, kw]
        out[:, kw * 128:(kw + 1) * 128] = blk
    return out


def _pack_h1(w):
    """h1 stride-2 dense+B+C packing (std layout), as in baseline."""
    dense = np.zeros((128, 3 * 128), np.float32)
    for kw in range(3):
        blk = np.zeros((128, 128), np.float32)
        for a in range(2):
            for b in range(2):
                kh = a - 2 * b + 1
                if 0 <= kh <= 2:
                    blk[a * 64:(a + 1) * 64, b * 64:(b + 1) * 64] = \
                        w[:, :, kh, kw].T
        dense[:, kw * 128:(kw + 1) * 128] = blk
    B = np.zeros((128, 3 * 64), np.float32)
    C = np.zeros((128, 3 * 64), np.float32)
    for kw in range(3):
        B[64:128, kw * 64:(kw + 1) * 64] = w[:, :, 0, kw].T
        C[0:64, kw * 64:(kw + 1) * 64] = w[:, :, 1, kw].T
        C[64:128, kw * 64:(kw + 1) * 64] = w[:, :, 2, kw].T
    return np.concatenate([dense, B, C], 1)      # [128, 768]


def _pack_im2col_w(w):
    out = np.zeros((64, 9 * 64), np.float32)
    for t in range(9):
        kh, kw = divmod(t, 3)
        out[:, t * 64:(t + 1) * 64] = w[:, :, kh, kw].T
    return out


def _pack_weights(inp):
    pk = {}
    # low0: [36, 128], rows 9*(rr+1)+3*kw+ch where rr in {-1..2} is the
    # input-row offset relative to 2j. Both output parities share the
    # same input-row streams (even out: kh=rr+1, odd out: kh=rr), so the
    # im2col needs only 12 gather DMAs per chunk.
    w36 = np.zeros((36, 128), np.float32)
    for o in range(128):
        op, oc = _zmap(o)
        for kh in range(3):
            for kw in range(3):
                rr = kh - 1 + op
                for ch in range(3):
                    w36[(rr + 1) * 9 + 3 * kw + ch, o] = \
                        inp["low0_w"][oc, ch, kh, kw]
    pk["w_low0"] = w36

    main = []
    for i in range(4):
        wf = np.zeros((64, 64, 3, 3), np.float32)
        wf[:61, :61] = inp["ll_rh_w"][i]
        wf[61:, 61:] = inp["ll_lh_w"][i]
        main.append(np.concatenate(
            [_pack_block(wf, _zmap, _zmap, "dense"),
             _pack_block(wf, _zmap, _zmap, "cross")], 1))
    whl0 = np.zeros((64, 64, 3, 3), np.float32)
    whl0[:, :61] = inp["hl0_w"]
    main.append(np.concatenate(
        [_pack_block(whl0, _zmap, _smap, "dense"),
         _pack_block(whl0, _zmap, _smap, "cross")], 1))
    main.append(_pack_h1(inp["hl_w"][0]))
    pk["w_main"] = np.concatenate(main, 1)       # [128, 6*768]

    pk["w_h2h3"] = np.concatenate(
        [_pack_im2col_w(inp["hl_w"][1]), _pack_im2col_w(inp["hl_w"][2])], 1)
    pk["w_lin"] = (inp["lin_w"].T / 64.0).astype(np.float32)
    selL = np.zeros((30, 20), np.float32)
    for q in range(30):
        for p in range(20):
            if q % 10 == FEAT_PERM[p % 10]:
                selL[q, p] = 1.0
    pk["w_sel"] = selL
    cmask = np.zeros((30, 6), np.float32)
    for q in range(30):
        for n in range(6):
            if q // 10 == n % 3:
                cmask[q, n] = 1.0
    pk["cmask"] = cmask
    pmask = np.zeros((20, 6), np.float32)
    for p in range(20):
        for n in range(6):
            if p // 10 == n // 3:
                pmask[p, n] = 1.0
    pk["pmask"] = pmask

    bias = np.zeros((128, 9), np.float32)
    llb = [np.concatenate([inp["ll_rh_b"][i], inp["ll_lh_b"][i]])
           for i in range(4)]
    for p in range(128):
        _, c = _zmap(p)
        bias[p, 0] = inp["low0_b"][c]
        for i in range(4):
            bias[p, 1 + i] = llb[i][c]
    bias[0:64, 5] = bias[64:128, 5] = inp["hl0_b"]
    bias[0:64, 6] = bias[64:128, 6] = inp["hl_b"][0]
    bias[0:64, 7] = inp["hl_b"][1]
    bias[0:64, 8] = inp["hl_b"][2]
    pk["bias"] = bias
    pk["lin_b"] = inp["lin_b"].reshape(30, 1).astype(np.float32)
    return pk


# ---------------------------------------------------------------------------
# device program
# ---------------------------------------------------------------------------

def _emit_fold_layer(nc, psum, src, S, dst, wts, bcol, bias_sb, mask_sb,
                     rng, kind, build_S):
    wd = [wts[:, k * 128:(k + 1) * 128] for k in range(3)]
    wx = [wts[:, 384 + k * 128:384 + (k + 1) * 128] for k in range(3)]
    odd_pending = None
    for batch in _batches(*rng):
        b0, n = batch[0], len(batch)
        acc = psum.tile([128, 4, 512], f32, name="acc4", tag="b")
        for kw in range(3):
            for i, j in enumerate(batch):
                nc.tensor.matmul(acc[:, i, :], wd[kw], src[:, j, kw:kw + 512],
                                 start=(kw == 0), stop=False)
        for kw in range(3):
            for i, j in enumerate(batch):
                nc.tensor.matmul(acc[:, i, :], wx[kw], S[:, j, kw:kw + 512],
                                 start=False, stop=(kw == 2))
        be = bias_sb[:, bcol:bcol + 1]
        dstb = dst[:, b0:b0 + n, 1:513]
        if kind == "ll":
            # relu+bias on all 128 slots for the whole batch; the lh
            # slots (32:35, 64:67, both at aligned PSUM starts) are then
            # overwritten with exact tanh segments, one scalar op each.
            nc.vector.tensor_scalar(dstb, acc[:, 0:n, :],
                                    be, 0.0, ALU.add, ALU.max)
            nc.scalar.activation(dst[32:35, b0:b0 + n, 1:513],
                                 acc[32:35, 0:n, :], AF.Tanh,
                                 bias=be[32:35])
            nc.scalar.activation(dst[64:67, b0:b0 + n, 1:513],
                                 acc[64:67, 0:n, :], AF.Tanh,
                                 bias=be[64:67])
        else:
            nc.vector.tensor_scalar(dst[0:64, b0:b0 + n, 1:513],
                                    acc[0:64, 0:n, :], be[0:64],
                                    None, ALU.add)
            nc.scalar.activation(dst[64:128, b0:b0 + n, 1:513],
                                 acc[64:128, 0:n, :], AF.Identity,
                                 bias=be[64:128])
        for j in batch:
            if j <= ZT or j >= ZB:
                nc.vector.tensor_scalar_mul(dst[:, j, 1:513],
                                            dst[:, j, 1:513],
                                            mask_sb[:, j:j + 1])
        if build_S:
            # Even half writes backward (b0-1..): never clobbers an S entry
            # a later cross matmul of THIS layer still needs. The odd half
            # writes forward (b0+1..b0+n), overlapping the next batch's
            # cross reads -- so it is emitted one batch late, after those
            # reads are in program order.
            nc.vector.tensor_copy(_flat(S, 0, 64, b0 - 1, n),
                                  _flat(dst, 0, 64, b0, n))
            if odd_pending is not None:
                p0, pn = odd_pending
                nc.vector.tensor_copy(_flat(S, 64, 64, p0 + 1, pn),
                                      _flat(dst, 64, 64, p0, pn))
            odd_pending = (b0, n)
    if build_S and odd_pending is not None:
        p0, pn = odd_pending
        nc.vector.tensor_copy(_flat(S, 64, 64, p0 + 1, pn),
                              _flat(dst, 64, 64, p0, pn))


def _emit_feats16(nc, bufA, xX, yB, half):
    """Build feats for one 16-sr half of the owned region in place in
    xX [20, 16*SLAB]: gather the X broadcast rows for the whole half,
    then per 8-sr chunk gather Y rows into the shared yB [20, 8*SLAB]
    scratch and multiply xX *= yB (ones rows pre-set at startup).
    X rows: r:0-3 g:4-6 b:7-8 (9=const); Y rows: r:{0} g:{1,4} b:{2,5,7}
    (3,6,8,9=const); +10 for the odd-row parity block."""
    s0 = OWN0 + half * 16
    XRUNS = [(0, 4), (4, 3), (7, 2)]
    YRUNS = [[0], [1, 4], [2, 5, 7]]
    ba = bufA[:]
    pstep = ba.ap[0][0]
    for par in range(2):
        qeng = nc.sync if par == 0 else nc.scalar
        for ci in range(3):
            sp = (32 if par == 0 else 64) + ci      # Z lh slots
            soff = ba.offset + sp * pstep + s0 * SLAB
            p0, n = XRUNS[ci]
            srcap = bass.AP(ba.tensor, soff,
                            [[pstep, 1], [0, n], [1, 16 * SLAB]])
            dstx = _ap(xX, 10 * par + p0, 0, [[1, n], [1, 16 * SLAB]])
            qeng.dma_start(dstx, srcap)
    for c8 in range(2):
        co = c8 * 8 * SLAB
        for par in range(2):
            qeng = nc.sync if par == 0 else nc.scalar
            for ci in range(3):
                sp = (32 if par == 0 else 64) + ci
                soff = ba.offset + sp * pstep + (s0 + 8 * c8) * SLAB
                for yp in YRUNS[ci]:
                    srcy = bass.AP(ba.tensor, soff,
                                   [[pstep, 1], [1, 8 * SLAB]])
                    dsty = _ap(yB, 10 * par + yp, 0,
                               [[1, 1], [1, 8 * SLAB]])
                    qeng.dma_start(dsty, srcy)
        xsl = _ap(xX, 0, co, [[1, 20], [1, 8 * SLAB]])
        nc.vector.tensor_mul(xsl, xsl, yB[:])


def _emit_tform(nc, psum, wm20, feats, outstage, out_d):
    """32 owned super-rows: 8 groups of 4 matmuls into the 4 banks of one
    PSUM tile, PSUM->SBUF copies alternating DVE/scalar, output DMAs
    alternating sync/scalar queues (one per parity per 8-sr sub)."""
    for sub in range(4):                       # 8 srs each
        for bi in range(2):                    # 4 srs each
            g0 = sub * 8 + bi * 4
            acc = psum.tile([6, 4, 512], f32, name="acc4", tag="b")
            for i in range(4):
                sr = g0 + i                    # 0..31, owned index
                half, off = divmod(sr, 16)
                fo = off * SLAB + 1
                nc.tensor.matmul(acc[:, i, :], wm20[:],
                                 feats[half][:, fo:fo + 512],
                                 start=True, stop=True)
            ost = _ap(outstage, 0, bi * 4 * 512, [[1, 6], [1, 4 * 512]])
            accf = _ap(acc, 0, 0, [[1, 6], [1, 4 * 512]])
            if bi == 0:
                nc.vector.tensor_copy(ost, accf)
            else:
                nc.scalar.activation(ost, accf, AF.Copy)
        r0 = 2 * sub * 8
        for par in range(2):
            sap = _ap(outstage, par * 3, 0, [[1, 3], [512, 8], [1, 512]])
            dap = bass.AP(out_d[:].tensor, (r0 + par) * 512,
                          [[64 * 512, 3], [2 * 512, 8], [1, 512]])
            (nc.sync if par == 0 else nc.scalar).dma_start(dap, sap)


def _build_program(debug=False):
    nc = bacc.Bacc("TRN2", target_bir_lowering=False, debug=False,
                   num_devices=NCORES)

    x_in = nc.dram_tensor("x", [3, LR, SLAB], bf16, kind="ExternalInput")
    wmain_in = nc.dram_tensor("w_main", [128, 6 * 768], bf16,
                              kind="ExternalInput")
    wlow0_in = nc.dram_tensor("w_low0", [36, 128], bf16, kind="ExternalInput")
    wh23_in = nc.dram_tensor("w_h2h3", [64, 1152], bf16, kind="ExternalInput")
    wsel_in = nc.dram_tensor("w_sel", [30, 20], bf16, kind="ExternalInput")
    cmask_in = nc.dram_tensor("cmask", [30, 6], f32, kind="ExternalInput")
    pmask_in = nc.dram_tensor("pmask", [20, 6], f32, kind="ExternalInput")
    wlin_in = nc.dram_tensor("w_lin", [64, 30], f32, kind="ExternalInput")
    bias_in = nc.dram_tensor("bias", [128, 9], f32, kind="ExternalInput")
    linb_in = nc.dram_tensor("lin_b", [30, 1], f32, kind="ExternalInput")
    maskI_in = nc.dram_tensor("mask_i", [128, NSR], f32, kind="ExternalInput")
    maskh1_in = nc.dram_tensor("mask_h1", [128, 20], f32, kind="ExternalInput")

    out_d = nc.dram_tensor("out", [3, 64, W], f32, kind="ExternalOutput")

    cc_in = nc.dram_tensor("cc_in", [64 * 4 * 32], bf16)
    cc_gath = nc.dram_tensor("cc_gath", [NCORES * 64 * 4 * 32], bf16,
                             addr_space="Shared")
    dbg = {}
    if debug:
        for nm, shp, dt in [("i0", [128, NSR * SLAB], bf16),
                            ("i1", [128, NSR * SLAB], bf16),
                            ("i2", [128, NSR * SLAB], bf16),
                            ("i3", [128, NSR * SLAB], bf16),
                            ("i4", [128, NSR * SLAB], bf16),
                            ("hl0", [128, NSR * SLAB], bf16),
                            ("h1", [128, 18 * 256], bf16),
                            ("pool1", [64, 20 * 130], bf16),
                            ("pool2", [64, 4 * 32], bf16),
                            ("wp", [30, 1], f32)]:
            dbg[nm] = nc.dram_tensor("dbg_" + nm, shp, dt, kind="ExternalOutput")

    with tile.TileContext(nc) as tc, ExitStack() as ctx:
        pers = ctx.enter_context(tc.tile_pool(name="pers", bufs=1))
        psum = ctx.enter_context(tc.tile_pool(name="psum", bufs=2, space="PSUM"))

        # persistent tiles
        w_low0 = pers.tile([36, 128], bf16)
        bias_sb = pers.tile([128, 9], f32)
        mask_sb = pers.tile([128, NSR], f32)
        w_main = pers.tile([128, 6 * 768], bf16)
        bufA = pers.tile([128, NSR, SLAB], bf16)
        bufB = pers.tile([128, NSR, SLAB], bf16)
        w_h2h3 = pers.tile([64, 1152], bf16)
        w_sel = pers.tile([30, 20], bf16)
        cmask_sb = pers.tile([30, 6], f32)
        pmask_sb = pers.tile([20, 6], f32)
        w_lin = pers.tile([64, 30], f32)
        linb_sb = pers.tile([30, 1], f32)
        maskh1_sb = pers.tile([128, 20], f32)

        # per-pixel feature staging: xA/xC hold X rows for halves 0/1 and
        # become feats in place; yB is 8-sr Y-row scratch shared by all
        # four chunks. Lives below sp/hlp/tfp on the pool stack so it
        # survives until the Tform.
        fpool = ctx.enter_context(tc.tile_pool(name="fpool", bufs=1))
        xA = fpool.tile([20, 16 * SLAB], bf16)
        yB = fpool.tile([20, 8 * SLAB], bf16)
        xC = fpool.tile([20, 16 * SLAB], bf16)

        sp_cm = tc.tile_pool(name="sp", bufs=1)
        sp = sp_cm.__enter__()
        Sbuf = sp.tile([128, NSR, SLAB], bf16)

        # ---- low0 via fold-K im2col: one [54->128] matmul per super-row.
        # Emit chunk-0 input DMAs first so the PE can start ASAP; split
        # the gather DMAs across the sync and scalar HWDGE queues.
        with tc.tile_pool(name="imcp", bufs=2) as imcp:
            first = True
            CHUNKS = [(1, 4), (5, 11), (16, 11), (27, 11), (38, 5)]
            for j0, ns in CHUNKS:
                imc = imcp.tile([36, 11 * 512], bf16, name="imc", tag="imc")
                for t in range(12):
                    rr, kw = divmod(t, 3)
                    src = bass.AP(x_in[:].tensor,
                                  (2 * j0 + rr - 1) * SLAB + kw,
                                  [[LR * SLAB, 3], [2 * SLAB, ns], [1, 512]])
                    qeng = nc.sync if t % 2 == 0 else nc.scalar
                    qeng.dma_start(imc[9 * rr + 3 * kw:9 * rr + 3 * kw + 3,
                                       0:ns * 512], src)
                if first:
                    # small weights on the HWDGE queues, bulk on gpsimd
                    nc.sync.dma_start(w_low0[:], wlow0_in[:])
                    nc.scalar.dma_start(bias_sb[:], bias_in[:])
                    nc.sync.dma_start(mask_sb[:], maskI_in[:])
                    nc.gpsimd.dma_start(w_main[:], wmain_in[:])
                    nc.gpsimd.dma_start(w_h2h3[:], wh23_in[:])
                    nc.gpsimd.dma_start(w_sel[:], wsel_in[:])
                    nc.gpsimd.dma_start(cmask_sb[:], cmask_in[:])
                    nc.gpsimd.dma_start(pmask_sb[:], pmask_in[:])
                    nc.gpsimd.dma_start(w_lin[:], wlin_in[:])
                    nc.gpsimd.dma_start(linb_sb[:], linb_in[:])
                    nc.gpsimd.dma_start(maskh1_sb[:], maskh1_in[:])
                    # zero pads (cols 0/513), S edge super-rows, and the
                    # ones-rows of the feature staging tiles
                    nc.gpsimd.memset(bufA[:, :, 0:1], 0.0)
                    nc.gpsimd.memset(bufA[:, :, 513:514], 0.0)
                    nc.gpsimd.memset(bufB[:, :, 0:1], 0.0)
                    nc.gpsimd.memset(bufB[:, :, 513:514], 0.0)
                    nc.gpsimd.memset(Sbuf[64:128, 0:2, :], 0.0)
                    nc.gpsimd.memset(xA[:], 1.0)
                    nc.gpsimd.memset(yB[:], 1.0)
                    nc.gpsimd.memset(xC[:], 1.0)
                    first = False
                for bt in _batches(j0, j0 + ns - 1):
                    b0, n = bt[0], len(bt)
                    acc = psum.tile([128, 4, 512], f32, name="acc4", tag="b")
                    for i, j in enumerate(bt):
                        si = j - j0
                        nc.tensor.matmul(acc[:, i, :], w_low0[:],
                                         imc[:, si * 512:(si + 1) * 512],
                                         start=True, stop=True)
                    be = bias_sb[:, 0:1]
                    # split the batch writeback by output-row parity:
                    # even rows on DVE, odd rows on the scalar engine.
                    for p0 in range(2):
                        i0 = (p0 - b0) % 2     # slice idx with parity p0
                        if i0 >= n:
                            continue
                        cnt = (n - i0 + 1) // 2
                        asl = _ap(acc, 0, i0 * 512,
                                  [[1, 128], [1024, cnt], [1, 512]])
                        dsl = _ap(bufA, 0, (b0 + i0) * SLAB + 1,
                                  [[1, 128], [2 * SLAB, cnt], [1, 512]])
                        if p0 == 0:
                            nc.vector.tensor_scalar(dsl, asl, be,
                                                    None, ALU.add)
                        else:
                            nc.scalar.activation(dsl, asl, AF.Identity,
                                                 bias=be)
                    for j in bt:
                        if j <= ZT or j >= ZB:
                            nc.vector.tensor_scalar_mul(
                                bufA[:, j, 1:513], bufA[:, j, 1:513],
                                mask_sb[:, j:j + 1])
                    nc.vector.tensor_copy(_flat(Sbuf, 0, 64, b0 - 1, n),
                                          _flat(bufA, 0, 64, b0, n))
                    nc.vector.tensor_copy(_flat(Sbuf, 64, 64, b0 + 1, n),
                                          _flat(bufA, 64, 64, b0, n))

        if debug:
            nc.sync.dma_start(dbg["i0"][:],
                              bufA[:].rearrange("p a b -> p (a b)"))

        # ---- ll layers + hl0 (S-trick fold layers) ----
        bufs = [bufA, bufB]
        for i in range(4):
            if i == 1:
                # ll1 reads S[0:64, 41] which must be zero (bufB sr 42
                # was never written); low0's build left stale data there.
                nc.gpsimd.memset(Sbuf[0:64, 41:42, :], 0.0)
            _emit_fold_layer(nc, psum, bufs[i % 2], Sbuf, bufs[(i + 1) % 2],
                             w_main[:, i * 768:(i + 1) * 768], 1 + i,
                             bias_sb, mask_sb, R_LL[i], "ll", True)
            if debug and i < 3:
                nc.sync.dma_start(
                    dbg[f"i{i + 1}"][:],
                    bufs[(i + 1) % 2][:].rearrange("p a b -> p (a b)"))

        # feats gathers/muls launched here: they only read bufA (final I)
        # and run on the DMA engines + DVE, overlapping hl0/h1/h2.
        _emit_feats16(nc, bufA, xA, yB, 0)
        _emit_feats16(nc, bufA, xC, yB, 1)

        _emit_fold_layer(nc, psum, bufA, Sbuf, bufB,
                         w_main[:, 4 * 768:5 * 768], 5,
                         bias_sb, mask_sb, R_HL0, "copy", False)
        sp_cm.__exit__(None, None, None)
        if debug:
            nc.sync.dma_start(dbg["i4"][:],
                              bufA[:].rearrange("p a b -> p (a b)"))
            nc.sync.dma_start(dbg["hl0"][:],
                              bufB[:].rearrange("p a b -> p (a b)"))

        with tc.tile_pool(name="hlp", bufs=1) as hlp:
            # ---- h1 (stride-2 fold conv from bufB, std layout) ----
            wh1 = w_main[:, 5 * 768:]
            wA = [wh1[:, k * 128:(k + 1) * 128] for k in range(3)]
            wB = [wh1[:, 384 + k * 64:384 + (k + 1) * 64] for k in range(3)]
            wC = [wh1[:, 576 + k * 64:576 + (k + 1) * 64] for k in range(3)]
            h1fold = hlp.tile([128, 18, 256], bf16)
            for batch in _batches(*R_H1):
                b0, n = batch[0], len(batch)
                acc = psum.tile([128, 4, 512], f32, name="acc4", tag="b")
                for kw in range(3):
                    for i, m in enumerate(batch):
                        nc.tensor.matmul(acc[:, i, 0:256], wA[kw],
                                         bufB[:, 2 * m, kw:kw + 512:2],
                                         start=(kw == 0), stop=False)
                for kw in range(3):
                    for i, m in enumerate(batch):
                        nc.tensor.matmul(acc[0:64, i, 0:256], wB[kw][64:128, :],
                                         bufB[64:128, 2 * m - 1, kw:kw + 512:2],
                                         start=False, stop=False,
                                         tile_position=(64, 0))
                    for i, m in enumerate(batch):
                        nc.tensor.matmul(acc[64:128, i, 0:256], wC[kw][:],
                                         bufB[:, 2 * m + 1, kw:kw + 512:2],
                                         start=False, stop=(kw == 2),
                                         tile_position=(0, 64))
                be = bias_sb[:, 6:7]
                aslE = _ap(acc, 0, 0, [[1, 64], [512, n], [1, 256]])
                aslO = _ap(acc, 64, 0, [[1, 64], [512, n], [1, 256]])
                dslE = _ap(h1fold, 0, (b0 - 2) * 256, [[1, 64], [1, n * 256]])
                dslO = _ap(h1fold, 64, (b0 - 2) * 256, [[1, 64], [1, n * 256]])
                nc.vector.tensor_scalar(dslE, aslE, be[0:64],
                                        0.0, ALU.add, ALU.max)
                nc.scalar.activation(dslO, aslO, AF.Relu, bias=be[64:128])
                for m in batch:
                    if m in (2, 3, 18, 19):
                        sl = h1fold[:, m - 2, :]
                        nc.vector.tensor_scalar_mul(sl[:], sl[:],
                                                    maskh1_sb[:, m:m + 1])
            if debug:
                nc.sync.dma_start(dbg["h1"][:],
                                  h1fold[:].rearrange("p a b -> p (a b)"))

            # pool1 via in-SBUF 2x2 max: horizontal pairs then the two
            # parity halves (partition-shifted operands).
            h1h = hlp.tile([128, 18, 128], bf16)
            nc.vector.tensor_max(h1h[:], h1fold[:, :, 0:256:2],
                                 h1fold[:, :, 1:256:2])
            h1v = hlp.tile([64, 18, 128], bf16)
            _h1h = h1h[:]
            nc.vector.tensor_copy(
                bass.AP(h1v[:].tensor, h1v[:].offset,
                        [[h1v[:].ap[0][0], 64], [1, 18 * 128]]),
                bass.AP(_h1h.tensor, _h1h.offset + 64 * _h1h.ap[0][0],
                        [[_h1h.ap[0][0], 64], [1, 18 * 128]]))
            pool1 = hlp.tile([64, 20, 130], bf16)
            nc.gpsimd.memset(pool1[:], 0.0)
            nc.vector.tensor_max(pool1[:, 2:20, 1:129],
                                 h1h[0:64, :, :], h1v[:])
            if debug:
                nc.sync.dma_start(dbg["pool1"][:],
                                  pool1[:].rearrange("p a b -> p (a b)"))

            # ---- h2 via im2col (9 taps, K=64) ----
            imc2 = hlp.tile([64, 9 * 512], bf16)
            for t in range(9):
                kh, kw = divmod(t, 3)
                src = _ap(pool1, 0, (2 + kh) * 130 + kw,
                          [[1, 64], [2 * 130, 8], [2, 64]])
                if t % 2 == 0:
                    nc.vector.tensor_copy(imc2[:, t * 512:(t + 1) * 512], src)
                else:
                    nc.scalar.activation(imc2[:, t * 512:(t + 1) * 512], src,
                                         AF.Copy)
            acc2 = psum.tile([64, 4, 512], f32, name="acc4", tag="b")
            for t in range(9):
                nc.tensor.matmul(acc2[:, 0, :], w_h2h3[:, t * 64:(t + 1) * 64],
                                 imc2[:, t * 512:(t + 1) * 512],
                                 start=(t == 0), stop=(t == 8))
            h2sb = hlp.tile([64, 8, 64], bf16)
            nc.scalar.activation(h2sb[:].rearrange("p a b -> p (a b)"),
                                 acc2[:, 0, :], AF.Relu, bias=bias_sb[0:64, 7:8])

            # pool2 -> cc_in
            tmp2 = hlp.tile([64, 8, 32], bf16)
            nc.vector.tensor_max(tmp2[:], h2sb[:, :, 0:64:2],
                                 h2sb[:, :, 1:64:2])
            pool2 = hlp.tile([64, 4, 32], bf16)
            nc.vector.tensor_max(pool2[:], tmp2[:, 0:8:2, :],
                                 tmp2[:, 1:8:2, :])
            nc.sync.dma_start(cc_in[:],
                              pool2[:].rearrange("p a b -> p (a b)"))
            if debug:
                nc.sync.dma_start(dbg["pool2"][:],
                                  pool2[:].rearrange("p a b -> p (a b)"))

        with tc.tile_pool(name="tfp", bufs=1) as tfp:
            outstage = tfp.tile([6, 8 * 512], f32)
            wm20 = tfp.tile([20, 6], bf16)
            p2f = tfp.tile([64, 34, 34], bf16)
            nc.gpsimd.memset(p2f[:], 0.0)

            # ---- AllGather pool2 ----
            with tc.tile_critical():
                cc_sem = nc.alloc_semaphore("cc_sem")
                nc.gpsimd.collective_compute(
                    "AllGather", ALU.bypass,
                    replica_groups=[list(range(NCORES))],
                    ins=[cc_in[:]], outs=[cc_gath[:]],
                ).then_inc(cc_sem)
                nc.gpsimd.wait_ge(cc_sem, 1)

            # ---- h3 tail (replicated) ----
            for q in range(NCORES):
                src = bass.AP(cc_gath[:].tensor, q * 64 * 4 * 32,
                              [[4 * 32, 64], [32, 4], [1, 32]])
                qeng = nc.sync if q % 2 == 0 else nc.scalar
                qeng.dma_start(p2f[:, 1 + 4 * q:5 + 4 * q, 1:33], src)
            imc3 = tfp.tile([64, 9 * 256], bf16)
            for t in range(9):
                kh, kw = divmod(t, 3)
                src = _ap(p2f, 0, kh * 34 + kw,
                          [[1, 64], [2 * 34, 16], [2, 16]])
                if t % 2 == 0:
                    nc.vector.tensor_copy(imc3[:, t * 256:(t + 1) * 256], src)
                else:
                    nc.scalar.activation(imc3[:, t * 256:(t + 1) * 256], src,
                                         AF.Copy)
            acc3 = psum.tile([64, 4, 512], f32, name="acc4", tag="b")
            for t in range(9):
                nc.tensor.matmul(acc3[:, 0, 0:256],
                                 w_h2h3[:, 576 + t * 64:576 + (t + 1) * 64],
                                 imc3[:, t * 256:(t + 1) * 256],
                                 start=(t == 0), stop=(t == 8))
            h3sb = tfp.tile([64, 16, 16], bf16)
            nc.scalar.activation(h3sb[:].rearrange("p a b -> p (a b)"),
                                 acc3[:, 0, 0:256], AF.Relu,
                                 bias=bias_sb[0:64, 8:9])
            tmp3 = tfp.tile([64, 16, 8], bf16)
            nc.vector.tensor_max(tmp3[:], h3sb[:, :, 0:16:2],
                                 h3sb[:, :, 1:16:2])
            h3p = tfp.tile([64, 8, 8], f32)
            nc.vector.tensor_max(h3p[:], tmp3[:, 0:16:2, :],
                                 tmp3[:, 1:16:2, :])
            gsum = tfp.tile([64, 1], f32)
            nc.vector.reduce_sum(gsum[:],
                                 h3p[:].rearrange("p a b -> p (a b)"),
                                 axis=mybir.AxisListType.X)
            accW = psum.tile([30, 4, 512], f32, name="acc4", tag="b")
            nc.tensor.matmul(accW[:, 0, 0:1], w_lin[:], gsum[:],
                             start=True, stop=True)
            wp_sb = tfp.tile([30, 1], f32)
            nc.scalar.activation(wp_sb[:], accW[:, 0, 0:1], AF.Identity,
                                 bias=linb_sb[:])
            if debug:
                nc.sync.dma_start(dbg["wp"][:], wp_sb[:])
            wpR = tfp.tile([30, 6], bf16)
            nc.vector.tensor_scalar_mul(wpR[:], cmask_sb[:], wp_sb[:])
            accM = psum.tile([20, 4, 512], f32, name="acc4", tag="b")
            nc.tensor.matmul(accM[:, 0, 0:6], w_sel[:], wpR[:],
                             start=True, stop=True)
            nc.vector.tensor_tensor(wm20[:], accM[:, 0, 0:6], pmask_sb[:],
                                    ALU.mult)

            _emit_tform(nc, psum, wm20, [xA, xC], outstage, out_d)

    nc.compile()
    return nc


# ---------------------------------------------------------------------------
# host entry
# ---------------------------------------------------------------------------

def kernel(**inputs):
    inp = {k: np.asarray(v) for k, v in inputs.items()}
    debug = bool(_cached.get("debug", False))
    key = ("nc", debug)
    if key not in _cached:
        _cached[key] = _build_program(debug=debug)
    nc = _cached[key]

    pk = _pack_weights(inp)
    x = np.asarray(inp["x"], np.float32)[0]

    shared = {
        "w_main": pk["w_main"].astype(nbf),
        "w_low0": pk["w_low0"].astype(nbf),
        "w_h2h3": pk["w_h2h3"].astype(nbf),
        "w_sel": pk["w_sel"].astype(nbf),
        "cmask": pk["cmask"],
        "pmask": pk["pmask"],
        "w_lin": pk["w_lin"],
        "bias": pk["bias"],
        "lin_b": pk["lin_b"],
    }
    in_maps = []
    par_col = (np.arange(128) // 64)[:, None]
    for c in range(NCORES):
        g0 = 64 * c - HALO
        xs = np.zeros((3, LR, SLAB), np.float32)
        lo, hi = max(0, -g0), min(LR, H - g0)
        xs[:, lo:hi, 1:513] = x[:, g0 + lo:g0 + hi, :]
        gI = g0 + 2 * np.arange(NSR)[None, :] + par_col
        maskI = ((gI >= 0) & (gI < H)).astype(np.float32)
        gh = 32 * c - 6 + 2 * np.arange(20)[None, :] + par_col
        maskh1 = ((gh >= 0) & (gh < 256)).astype(np.float32)
        im = dict(shared)
        im["x"] = xs.astype(nbf)
        im["mask_i"] = maskI
        im["mask_h1"] = maskh1
        in_maps.append(im)

    res = run_bass_kernel_spmd(nc, in_maps, list(range(NCORES)))
    _cached["last_results"] = res
    out = np.concatenate([res.results[c]["out"] for c in range(NCORES)], axis=1)
    return out[None].astype(np.float32)
